# revision 1
# baseline (speedup 1.0000x reference)
"""Hawk (RG-LRU) Trainium2 kernel — sequence-sharded variant.

Sharding (8 cores): core = 2n + s -> batch n in 0..3, time-half s in {0,1}.
Each core computes ALL 1536 channels over its 1024-step half. No duplicated
mm1 work (the batch x channel sharding recomputes the full conv input on
both pair cores); instead the recurrence carry h[T/2-1] crosses the pair via
a masked AllReduce of a [128,12] vector, and the second half corrects
h += cumprod(alpha) * carry. The causal-conv halo is handled by giving each
core a 1028-col x window (first core: 4 zero cols; second: 4 halo cols).

All-SPMD: the two halves run the identical program; asymmetry (send/use of
the carry) comes from per-core 0/1 mask inputs.
"""

import numpy as np

import concourse.bacc as bacc
import concourse.mybir as mybir
import concourse.tile as tile
from concourse.bass_utils import run_bass_kernel_spmd

f32 = mybir.dt.float32
f32r = mybir.dt.float32r
bf16 = mybir.dt.bfloat16
AF = mybir.ActivationFunctionType
ALU = mybir.AluOpType


def build_nc(D, HID, T, num_cores=8):
    KD, KH = D // 128, HID // 128      # 8, 12
    MH = 2 * KH                        # 12 xh tiles + 12 gate tiles
    MD = D // 128
    TL = T // 2                        # local timesteps (1024)
    TW = TL + 4                        # x window incl pad/halo (1028)

    nc = bacc.Bacc("TRN2", target_bir_lowering=False, debug=False,
                   num_devices=num_cores)

    xt_d = nc.dram_tensor("xt", [128, KD, TW], bf16, kind="ExternalInput")
    w1_d = nc.dram_tensor("w1", [MH, 128, KD * 128], bf16, kind="ExternalInput")
    cw_d = nc.dram_tensor("cw", [128, KH, 4], f32, kind="ExternalInput")
    cb_d = nc.dram_tensor("cb", [128, KH], f32, kind="ExternalInput")
    w2_d = nc.dram_tensor("w2", [2 * KH, 128, KH * 128], bf16, kind="ExternalInput")
    gbh_d = nc.dram_tensor("gbh", [128, 2 * KH], f32, kind="ExternalInput")
    pch_d = nc.dram_tensor("pch", [128, KH], f32, kind="ExternalInput")
    w3_d = nc.dram_tensor("w3", [KH, 128, D], bf16, kind="ExternalInput")
    msk_d = nc.dram_tensor("msk", [128, 2], f32, kind="ExternalInput")
    out_d = nc.dram_tensor("o", [D, TL], bf16, kind="ExternalOutput")

    with tile.TileContext(nc) as tc:
        consts = tc.alloc_tile_pool(name="consts", bufs=1)
        gep = tc.alloc_tile_pool(name="ge", bufs=KH)
        w3p = tc.alloc_tile_pool(name="w3", bufs=KH)
        xhp = tc.alloc_tile_pool(name="xh", bufs=KH)
        xip = tc.alloc_tile_pool(name="xi", bufs=KH)
        ppp = tc.alloc_tile_pool(name="pp", bufs=KH)
        gpp = tc.alloc_tile_pool(name="gp", bufs=4)
        carp = tc.alloc_tile_pool(name="car", bufs=2)
        dramp = tc.alloc_tile_pool(name="dram", bufs=4, space="DRAM")
        w2p = tc.alloc_tile_pool(name="w2", bufs=4)
        xtp = tc.alloc_tile_pool(name="xt", bufs=1)

        cin1_b = dramp.tile([128, 8], f32)
        cout1_b = dramp.tile([128, 8], f32)
        cin2_b = dramp.tile([128, 4], f32)
        cout2_b = dramp.tile([128, 4], f32)
        RG = [[2 * i, 2 * i + 1] for i in range(num_cores // 2)]

        xt = xtp.tile([128, KD, TW], bf16, tag="xt")
        w1p = tc.alloc_tile_pool(name="w1", bufs=3)
        w1head = []

        nc.sync.dma_start(xt[:, :, 0:257], xt_d[:, :, 0:257])
        w1m0 = w1p.tile([128, KD, 128], bf16, tag="w1")
        nc.sync.dma_start(w1m0[:], w1_d[0].rearrange("p (k f) -> p k f", k=KD))
        w1head.append(w1m0)
        nc.sync.dma_start(xt[:, :, 257:514], xt_d[:, :, 257:514])
        for m in range(1, 3):
            w1m = w1p.tile([128, KD, 128], bf16, tag="w1")
            nc.sync.dma_start(
                w1m[:], w1_d[m].rearrange("p (k f) -> p k f", k=KD))
            w1head.append(w1m)
        nc.sync.dma_start(xt[:, :, 514:771], xt_d[:, :, 514:771])
        nc.sync.dma_start(xt[:, :, 771:TW], xt_d[:, :, 771:TW])

        cw = consts.tile([128, KH, 4], f32, tag="cw")
        nc.sync.dma_start(cw[:], cw_d[:])
        cb = consts.tile([128, KH], f32, tag="cb")
        nc.sync.dma_start(cb[:], cb_d[:])
        gbh = consts.tile([128, 2 * KH], f32, tag="gbh")
        nc.sync.dma_start(gbh[:], gbh_d[:])
        pch = consts.tile([128, KH], f32, tag="pch")
        nc.sync.dma_start(pch[:], pch_d[:])
        msk = consts.tile([128, 2], f32, tag="msk")
        nc.sync.dma_start(msk[:], msk_d[:])
        qrt = consts.tile([128, 1], f32, tag="qrt")
        nc.gpsimd.memset(qrt[:], 0.25)
        pc2 = consts.tile([128, KH], f32, tag="pc2")
        nc.vector.tensor_scalar(pc2[:], pch[:], 2.0, 0.0, ALU.mult, ALU.add)

        ge = [gep.tile([128, TL], bf16, tag="ge", name=f"ge{g}")
              for g in range(KH)]
        xh = [xhp.tile([128, TW], bf16, tag="xh", name=f"xh{m}")
              for m in range(KH)]
        # h_local per channel tile (kept until the cross-pair correction)
        xi = [xip.tile([128, TL], bf16, tag="xi", name=f"xi{p}")
              for p in range(KH)]
        # cumprod(alpha) per channel tile
        pp = [ppp.tile([128, TL], bf16, tag="pp", name=f"pp{p}")
              for p in range(KH)]
        gp = [gpp.tile([128, TL], bf16, tag="gp", name=f"gp{i}")
              for i in range(4)]
        carr = carp.tile([128, KH], f32, tag="car", name="carr")
        gat = carp.tile([128, KH], f32, tag="car", name="gat")

        # ---------------- Phase A: mm1 + conv + gelu(gate) ----------------
        with (
            tc.tile_pool(name="accv", bufs=2) as accvp,
            tc.tile_pool(name="psA", bufs=4, space="PSUM") as psa,
        ):
            for m in range(MH):
                if m < 3:
                    w1m = w1head[m]
                else:
                    w1m = w1p.tile([128, KD, 128], bf16, tag="w1")
                    nc.sync.dma_start(
                        w1m[:], w1_d[m].rearrange("p (k f) -> p k f", k=KD))
                if m < KH:
                    chunks = [(s, 257) for s in range(0, TW, 257)]
                else:
                    chunks = [(4, 512), (516, 512)]
                for s, CA in chunks:
                    ps = psa.tile([128, 512], f32)
                    for k in range(KD):
                        nc.tensor.matmul(
                            ps[:, 0:CA],
                            w1m[:, k, :],
                            xt[:, k, s:s + CA],
                            start=(k == 0),
                            stop=(k == KD - 1),
                        )
                    if m < KH:
                        nc.scalar.copy(xh[m][:, s:s + CA], ps[:, 0:CA])
                    else:
                        nc.scalar.activation(ge[m - KH][:, s - 4:s - 4 + CA],
                                             ps[:, 0:CA], AF.Gelu)
                if m < KH:
                    # causal depthwise conv: out[t] = sum_s w[s]*raw[t-3+s],
                    # valid for all TL outputs thanks to the 4-col window pad
                    acc = accvp.tile([128, TL], bf16, tag="acc")
                    nc.vector.tensor_scalar(
                        acc[:], xh[m][:, 1:1 + TL],
                        cw[:, m, 0:1], cb[:, m:m + 1],
                        ALU.mult, ALU.add)
                    for tap in (1, 2):
                        nc.vector.scalar_tensor_tensor(
                            acc[:], xh[m][:, 1 + tap:1 + tap + TL],
                            cw[:, m, tap:tap + 1],
                            acc[:], ALU.mult, ALU.add)
                    nc.vector.scalar_tensor_tensor(
                        xh[m][:, 4:4 + TL], xh[m][:, 4:4 + TL],
                        cw[:, m, 3:4], acc[:], ALU.mult, ALU.add)
        w1p.release()
        xtp.release()

        # ---------------- Phase B: mm2 + gates + local scan ----------------
        psbi = tc.alloc_tile_pool(name="psBi", bufs=1, space="PSUM")
        with (
            tc.tile_pool(name="alp", bufs=2) as alp,
            tc.tile_pool(name="bsc", bufs=2) as bscp,
            tc.tile_pool(name="tip", bufs=2) as tip,
            tc.tile_pool(name="psBf", bufs=1, space="PSUM") as psbf,
        ):
            w2head = []
            for g in (0, KH):
                w2g = w2p.tile([128, KH, 128], bf16, tag="w2")
                nc.sync.dma_start(
                    w2g[:], w2_d[g].rearrange("p (k f) -> p k f", k=KH))
                w2head.append(w2g)
            w3 = []
            for k in range(KH):
                w3k = w3p.tile([128, D], bf16, tag="w3", name=f"w3_{k}")
                nc.sync.dma_start(w3k[:], w3_d[k])
                w3.append(w3k)

            for p in range(KH):
                if p == 0:
                    w2f = w2head[0]
                else:
                    w2f = w2p.tile([128, KH, 128], bf16, tag="w2")
                    nc.sync.dma_start(
                        w2f[:], w2_d[p].rearrange("p (k f) -> p k f", k=KH))
                psf = psbf.tile([128, TL], f32, tag="psBf")
                for k in range(KH):
                    for h in range(2):
                        hs = h * 512
                        nc.tensor.matmul(
                            psf[:, hs:hs + 512],
                            w2f[:, k, :],
                            xh[k][:, 4 + hs:4 + hs + 512],
                            start=(k == 0),
                            stop=(k == KH - 1),
                        )
                alpha = alp.tile([128, TL], f32, tag="alp")
                bsc = bscp.tile([128, TL], f32, tag="bsc")
                if p == KH - 1:
                    # carry-critical tile: tanh kept in bsc so alpha^2 comes
                    # from a second Exp on ACT (no engine hop), shortening
                    # tanh->alpha->beta->bxh before the final stt+scan
                    nc.scalar.activation(bsc[:], psf[:], AF.Tanh,
                                         bias=gbh[:, p:p + 1], scale=0.5)
                    nc.scalar.activation(alpha[:], bsc[:], AF.Exp,
                                         bias=pch[:, p:p + 1],
                                         scale=pch[:, p:p + 1])
                    nc.scalar.activation(bsc[:], bsc[:], AF.Exp,
                                         bias=pc2[:, p:p + 1],
                                         scale=pc2[:, p:p + 1])
                else:
                    nc.scalar.activation(alpha[:], psf[:], AF.Tanh,
                                         bias=gbh[:, p:p + 1], scale=0.5)
                    nc.scalar.activation(alpha[:], alpha[:], AF.Exp,
                                         bias=pch[:, p:p + 1],
                                         scale=pch[:, p:p + 1])
                # pp = use_mask * cumprod(alpha): seeding the scan with the
                # mask makes pp identically 0 on even cores, so the carry
                # correction needs no separate masking of the gathered value
                nc.vector.tensor_tensor_scan(
                    pp[p][:, 0:512], alpha[:, 0:512], alpha[:, 0:512],
                    msk[:, 1:2], ALU.mult, ALU.bypass)
                nc.vector.tensor_tensor_scan(
                    pp[p][:, 512:TL], alpha[:, 512:TL], alpha[:, 512:TL],
                    pp[p][:, 511:512], ALU.mult, ALU.bypass)
                if p != KH - 1:
                    nc.vector.tensor_mul(bsc[:], alpha[:], alpha[:])
                nc.scalar.activation(bsc[:], bsc[:], AF.Sqrt,
                                     bias=qrt[:, 0:1], scale=-0.25)
                nc.vector.tensor_mul(bsc[:], bsc[:], xh[p][:, 4:4 + TL])
                if p == 0:
                    w2i = w2head[1]
                else:
                    w2i = w2p.tile([128, KH, 128], bf16, tag="w2")
                    nc.sync.dma_start(
                        w2i[:], w2_d[KH + p].rearrange("p (k f) -> p k f",
                                                       k=KH))
                psi = psbi.tile([128, TL], f32, tag="psBi")
                # last tile runs chunk-OUTER: its first 512 columns finish
                # ~2.6us before B's end, so the carry-critical ti/stt/scan
                # chain overlaps the remaining matmuls instead of trailing
                # them, pulling collective #2's launch forward
                if p == KH - 1:
                    for h in range(2):
                        hs = h * 512
                        for k in range(KH):
                            nc.tensor.matmul(
                                psi[:, hs:hs + 512],
                                w2i[:, k, :],
                                xh[k][:, 4 + hs:4 + hs + 512],
                                start=(k == 0),
                                stop=(k == KH - 1),
                            )
                else:
                    for k in range(KH):
                        for h in range(2):
                            hs = h * 512
                            nc.tensor.matmul(
                                psi[:, hs:hs + 512],
                                w2i[:, k, :],
                                xh[k][:, 4 + hs:4 + hs + 512],
                                start=(k == 0),
                                stop=(k == KH - 1),
                            )
                ti = tip.tile([128, TL], bf16, tag="tip")
                echunks = (512, 512)
                s = 0
                for CE in echunks:
                    sl = slice(s, s + CE)
                    nc.scalar.activation(ti[:, sl], psi[:, sl], AF.Tanh,
                                         bias=gbh[:, KH + p:KH + p + 1],
                                         scale=0.5)
                    nc.vector.scalar_tensor_tensor(
                        xi[p][:, sl], ti[:, sl], 1.0, bsc[:, sl],
                        ALU.add, ALU.mult)
                    nc.vector.tensor_tensor_scan(
                        xi[p][:, sl], alpha[:, sl], xi[p][:, sl],
                        0.0 if s == 0 else xi[p][:, s - 1:s],
                        ALU.mult, ALU.add)
                    s += CE
                # local end-state -> carry slot (masked to 0 on odd cores).
                # On ACT, not DVE: the collective's bounce-in DMA waits on
                # this engine's counter, and DVE's is polluted by the
                # correction ops scheduled ahead of it.
                nc.scalar.activation(carr[:, p:p + 1], xi[p][:, TL - 1:TL],
                                     AF.Copy, scale=msk[:, 0:1])

                # collective #1 (carries p0..7) launches mid-loop so its
                # ~30us runs under B's p8..11 matmuls. Only the issue +
                # in-bounce DMA go here; the readback and corrections are
                # deferred past the loop so no engine queue blocks on the
                # collective while B still has work.
                if p == 7:
                    nc.gpsimd.dma_start(cin1_b[:], carr[:, 0:8])
                    nc.gpsimd.collective_compute(
                        "AllReduce", ALU.add, replica_groups=RG,
                        ins=[cin1_b.opt()], outs=[cout1_b.opt()])
                if p >= 8:
                    # pre-products for the 1-op late correction; gp must be
                    # taken from ge before ge absorbs h_local
                    nc.vector.tensor_mul(gp[p - 8][:], ge[p][:], pp[p][:])
                    nc.vector.tensor_mul(ge[p][:], ge[p][:], xi[p][:])

        # collective #2 (p8..11) — issued at B's end, exposed; phase C's
        # k0..7 prefix contraction runs while it is in flight
        nc.gpsimd.dma_start(gat[:, 0:8], cout1_b[:])
        nc.sync.dma_start(cin2_b[:], carr[:, 8:12])
        nc.gpsimd.collective_compute(
            "AllReduce", ALU.add, replica_groups=RG,
            ins=[cin2_b.opt()], outs=[cout2_b.opt()])

        for q in range(8):
            nc.vector.scalar_tensor_tensor(
                xi[q][:], pp[q][:], gat[:, q:q + 1], xi[q][:],
                ALU.mult, ALU.add)
            nc.vector.tensor_mul(ge[q][:], ge[q][:], xi[q][:])
        nc.sync.dma_start(gat[:, 8:12], cout2_b[:])
        # late tiles: ge*h_local and ge*cumprod were pre-multiplied in B's
        # slack (gp tiles), so each correction is ONE stt on the critical
        # join: ge_final = (ge*pp)*carry + ge*h_local. Chunked in halves —
        # the dep tracker is slice-accurate, so pass-2's first k8 matmul
        # starts after half a correction instead of a full one.
        for q in (8, 9, 10, 11):
            for cs in (0, 512):
                sl = slice(cs, cs + 512)
                nc.vector.scalar_tensor_tensor(
                    ge[q][:, sl], gp[q - 8][:, sl], gat[:, q:q + 1],
                    ge[q][:, sl], ALU.mult, ALU.add)
        psbi.release()

        # ---------------- Phase C: mm3 ----------------
        # 4 PSUM tiles live at once; contract k0..9 for the whole group
        # first so the exposed p10/p11 collective hides under it
        with (
            tc.tile_pool(name="prt", bufs=MD) as prtp,
            tc.tile_pool(name="outp", bufs=4) as outp,
            tc.tile_pool(name="psC", bufs=4, space="PSUM") as psc,
        ):
            # two-pass: every output tile pre-accumulates its k0..7 prefix
            # (only early-corrected ge needed) and spills the partial to
            # SBUF, freeing the PSUM banks. This doubles the PE work that
            # can run while collective #2 + late corrections are in flight
            # (27us of cover vs the 14us an 8-bank PSUM allows directly).
            parts = []
            for m in range(MD):
                ps = psc.tile([128, TL], f32, tag="psC", name=f"psC_{m}")
                part = prtp.tile([128, TL], f32, tag="prt", name=f"prt_{m}")
                parts.append(part)
                for k in range(8):
                    for cq in range(2):
                        cs = cq * 512
                        nc.tensor.matmul(
                            ps[:, cs:cs + 512],
                            w3[k][:, m * 128:(m + 1) * 128],
                            ge[k][:, cs:cs + 512],
                            start=(k == 0),
                            stop=(k == 7),
                        )
                nc.scalar.copy(part[:, 0:512], ps[:, 0:512])
                nc.vector.tensor_copy(part[:, 512:TL], ps[:, 512:TL])
            for m in range(MD):
                ps = psc.tile([128, TL], f32, tag="psC", name=f"psC2_{m}")
                ot = outp.tile([128, TL], bf16, tag="outp", name=f"ot_{m}")
                for k in (8, 9, 10, 11):
                    for cq in range(2):
                        cs = cq * 512
                        nc.tensor.matmul(
                            ps[:, cs:cs + 512],
                            w3[k][:, m * 128:(m + 1) * 128],
                            ge[k][:, cs:cs + 512],
                            start=(k == 8),
                            stop=(k == KH - 1),
                        )
                for cq in range(2):
                    cs = cq * 512
                    sl = slice(cs, cs + 512)
                    nc.vector.tensor_add(ot[:, sl], parts[m][:, sl],
                                         ps[:, sl])
                    nc.sync.dma_start(out_d[m * 128:(m + 1) * 128, sl],
                                      ot[:, sl])
        w2p.release()
        dramp.release()
        carp.release()
        gpp.release()
        ppp.release()
        xip.release()
        xhp.release()
        w3p.release()
        gep.release()
        consts.release()

    nc.compile()
    return nc


def make_in_maps(x, input_w, conv_w, conv_b, gates_w, gates_b, forget_base,
                 output_w, D, HID, T, num_cores):
    KD, KH = D // 128, HID // 128
    N = x.shape[0]
    TL = T // 2
    np_bf16 = mybir.dt.np(bf16)

    # core-independent weight prep (natural channel order, no permutation)
    MH = 2 * KH
    w1sel = np.concatenate([input_w[HID:2 * HID], input_w[0:HID]], 0)
    w1T = w1sel.T  # [D, 2*HID]
    w1 = np.stack([
        np.ascontiguousarray(
            w1T[:, m * 128:(m + 1) * 128].reshape(KD, 128, 128)
            .transpose(1, 0, 2)).reshape(128, KD * 128)
        for m in range(MH)
    ]).astype(np_bf16)

    cw = np.ascontiguousarray(
        conv_w[:, 0, :].reshape(KH, 128, 4).transpose(1, 0, 2)
    ).astype(np.float32)
    cb = np.ascontiguousarray(conv_b.reshape(KH, 128).T).astype(np.float32)

    w2T = gates_w.T  # [HID, 2*HID]
    w2 = np.stack([
        np.ascontiguousarray(
            w2T[:, g * 128:(g + 1) * 128].reshape(KH, 128, 128)
            .transpose(1, 0, 2)).reshape(128, KH * 128)
        for g in range(2 * KH)
    ]).astype(np_bf16)

    gbt = np.ascontiguousarray(
        (0.5 * gates_b).reshape(2 * KH, 128).T).astype(np.float32)
    pcv = (-4.0 * np.log1p(np.exp(forget_base.astype(np.float64))))
    pct = np.ascontiguousarray(pcv.reshape(KH, 128).T).astype(np.float32)
    w3 = np.ascontiguousarray(
        output_w.T.reshape(KH, 128, D)).astype(np_bf16)

    in_maps = []
    for core in range(num_cores):
        n, s = core // 2, core % 2
        if s == 0:
            win = np.concatenate(
                [np.zeros((4, D), np.float32), x[n, 0:TL]], 0)
        else:
            win = x[n, TL - 4:T]
        xt = np.ascontiguousarray(
            win.T.reshape(KD, 128, TL + 4).transpose(1, 0, 2)).astype(np_bf16)
        msk = np.zeros((128, 2), np.float32)
        msk[:, 0] = 1.0 - s   # send mask (even half contributes its carry)
        msk[:, 1] = float(s)  # use mask (odd half applies the carry)
        in_maps.append({
            "xt": xt, "w1": w1, "cw": cw, "cb": cb, "w2": w2,
            "gbh": gbt, "pch": pct, "w3": w3, "msk": msk,
        })
    return in_maps


_CACHE = {}
TRACE = False
LAST_RES = None


def _get_nc(D, HID, T, num_cores):
    key = (D, HID, T, num_cores)
    if key not in _CACHE:
        _CACHE[key] = build_nc(D, HID, T, num_cores)
    return _CACHE[key]


def run_hawk(x, input_w, conv_w, conv_b, gates_w, gates_b, forget_base,
             output_w, num_cores=8):
    N, T, D = x.shape
    HID = input_w.shape[0] // 2
    nc = _get_nc(D, HID, T, num_cores)
    in_maps = make_in_maps(x, input_w, conv_w, conv_b, gates_w, gates_b,
                           forget_base, output_w, D, HID, T, num_cores)
    global LAST_RES
    res = run_bass_kernel_spmd(nc, in_maps, core_ids=list(range(num_cores)),
                               trace=TRACE)
    LAST_RES = res
    out = np.stack([
        np.concatenate([res.results[2 * n]["o"].astype(np.float32).T,
                        res.results[2 * n + 1]["o"].astype(np.float32).T], 0)
        for n in range(N)
    ])
    return np.ascontiguousarray(out.astype(np.float32))


def kernel(x, input_w, conv_w, conv_b, gates_w, gates_b, forget_base,
           output_w):
    return run_hawk(
        np.asarray(x, dtype=np.float32),
        np.asarray(input_w, dtype=np.float32),
        np.asarray(conv_w, dtype=np.float32),
        np.asarray(conv_b, dtype=np.float32),
        np.asarray(gates_w, dtype=np.float32),
        np.asarray(gates_b, dtype=np.float32),
        np.asarray(forget_base, dtype=np.float32),
        np.asarray(output_w, dtype=np.float32),
    )

